# revision 6
# baseline (speedup 1.0000x reference)
"""Trainium2 Bass kernel for AdaConv2d (instance-norm + per-sample dynamic
depthwise 3x3 conv + per-channel scale/bias + shared dense 3x3 conv, reflect
padding everywhere).

Data-parallel over batch: 8 samples -> 8 NeuronCores, one sample per core.

Math (per sample, per channel c):
    xn   = (x - mu_c) * rsqrt(var_c + eps)
    mid  = wp_c * depthwise3x3(reflect_pad(xn); ws_c) + b_c
         = v_c + b_c    with v = a*dw(x) - a*mu*s9,  a = wp*rsqrt(var+eps)
    out  = dense3x3(reflect_pad(mid); conv_w) + conv_b
         = dense3x3(reflect_pad(v); conv_w) + S@b + conv_b
      where S[co,ci] = sum_taps conv_w  (reflect padding makes the per-channel
      constant b contribute exactly S@b at every output pixel).

The v split is what makes fp8 viable: v has std ~0.0075 (vs mid's ~0.05
dominated by the per-channel constant b), so quantizing v*2^9 to fp8e4m3
keeps the dense-conv error ~1e-2 relative (gate is 2e-2).  The dense conv
runs as fp8 DoubleRow matmuls: each matmul contracts 256 = 2 channel tiles
(the pair dim of the moving AP strides between two v images that live in
one SBUF tile), at ~1.7x the bf16 rate.  The constant S@b + conv_b is
computed exactly in fp32 on the gpsimd engine (elementwise mult with a
host-broadcast b row + row-reduce) and added as the eviction bias.

Engine split: depthwise for tiles 2,3 on the tensor engine (diagonal fp8
weights, DoubleRow pairing two taps per matmul via a stride-2 pair dim);
tiles 0,1 on the vector engine (tensor_scalar 4x + tensor_add 2x ladder;
the 2-byte-misaligned dx==1 taps run as gpsimd scaled copies).  x arrives
fp8 for all tiles (halves the DMA) and is upconverted to fp16 on gpsimd
for the vector-engine tiles.  All stored images are [66 rows x 72 cols]
so the DoubleRow pair strides stay 16-byte aligned.
"""

import os
import sys
import types

import numpy as np
import ml_dtypes

B, C, H, W = 8, 512, 64, 64
KS = 3
EPS = 1e-5
N_CORES = 8
P = 128
CT = C // P            # 4 channel tiles
PADH = H + 2           # 66
PADW = W + 2           # 66
IMGC = 72              # stored row stride (16B-aligned images: 66*72 = 4752)
IMG = PADH * IMGC      # 4752
HW = H * W             # 4096
NCHUNK = 8             # 8-row output chunks -> one psum bank each
PE_TILES = (2, 3)      # depthwise on TensorE (diag fp8 DoubleRow matmuls)
DVE_TILES = (0, 1)     # depthwise on VectorE
CI_PAIRS = ((0, 1), (2, 3))
PAIR_ORDER = (1, 0)    # dense contraction: PE-tile pair first (ready early)
K_SC = 9               # v scale 2^9
M_SC = 9               # dense weight scale 2^9
S_SC = 9               # depthwise diag weight scale 2^9
OUT_SCALE = float(2.0 ** (-(K_SC + M_SC)))
# depthwise tap blocks for the PE tiles: 4 DoubleRow pairs + 1 single
DW_BLOCKS = (((0, 0), (0, 2)), ((1, 0), (1, 2)), ((2, 0), (2, 2)),
             ((0, 1), (2, 1)), ((1, 1),))
GP_TAPS = (1, 4, 7)    # dx==1 taps of the DVE ladder, done as gpsimd copies

E4 = ml_dtypes.float8_e4m3


def _install_ntff_hook():
    """Register the NTFF profiling hook that concourse expects under axon
    (missing antenv.axon_hooks module in this image)."""
    if "antenv.axon_hooks" in sys.modules:
        return
    try:
        mod = types.ModuleType("antenv.axon_hooks")
        holder = [None]
        mod.set_axon_ntff_profile_hook = lambda h: holder.__setitem__(0, h)
        mod.get_axon_ntff_profile_hook = lambda: holder[0]
        sys.modules["antenv.axon_hooks"] = mod
        from trn_agent_boot.trn_boot import _ntff_profile_via_ctypes

        hook = _ntff_profile_via_ctypes("/opt/axon/libaxon_pjrt.so")
        mod.set_axon_ntff_profile_hook(hook)
    except Exception:
        sys.modules.pop("antenv.axon_hooks", None)


_TRACE = os.environ.get("BASS_KERNEL_TRACE") == "1"
if _TRACE:
    _install_ntff_hook()

import concourse.tile as tile
from concourse import bacc, mybir
import concourse.bass_utils as bass_utils
from concourse.bass_utils import run_bass_kernel_spmd
from concourse.ap import AP

if _TRACE:
    bass_utils.upload_artifacts = lambda d: d

LAST_EXEC_NS = None
_CACHE = {}


def _taps():
    for tap in range(KS * KS):
        yield tap, tap // KS, tap % KS


def _reflect_borders(nc, img3):
    """Fill the 1-wide reflect border of a [128, 66, 72] image whose
    interior [1:65, 1:65] is already populated (cols first, then full rows
    so the corners come out as reflect-of-reflect, matching np.pad)."""
    nc.vector.tensor_copy(img3[:, 1:H + 1, 0:1], img3[:, 1:H + 1, 2:3])
    nc.vector.tensor_copy(img3[:, 1:H + 1, PADW - 1:PADW],
                          img3[:, 1:H + 1, PADW - 3:PADW - 2])
    nc.vector.tensor_copy(img3[:, 0:1, 0:PADW], img3[:, 2:3, 0:PADW])
    nc.vector.tensor_copy(img3[:, PADH - 1:PADH, 0:PADW],
                          img3[:, PADH - 3:PADH - 2, 0:PADW])


def _dedup_ldweights(nc):
    """Drop InstLdweights whose weights AP is identical to the previous
    weight load on the PE stream (bacc splits every matmul into LDW+MM;
    with one weight block reused across 8 chunk matmuls, 7 of 8 loads are
    redundant and serialize with the matmuls).  LDWs carrying semaphore
    waits/updates are kept."""
    n_removed = 0
    for f in nc.m.functions:
        for bb in f.blocks:
            insts = bb.instructions
            keep = []
            last_key = None
            for inst in insts:
                tn = type(inst).__name__
                if tn == "InstLdweights":
                    si = inst.sync_info
                    has_sync = si is not None and (
                        len(si.on_wait) > 0 or len(si.on_update) > 0
                    )
                    key = repr(inst.ins[0])
                    if key == last_key and not has_sync:
                        n_removed += 1
                        continue
                    last_key = key
                elif tn == "InstMatmult":
                    if getattr(inst, "is_transpose", False):
                        last_key = None
                keep.append(inst)
            if len(keep) != len(insts):
                bb.instructions = keep
    return n_removed


def _build():
    nc = bacc.Bacc("TRN2", target_bir_lowering=False, debug=False,
                   num_devices=N_CORES)
    f32 = mybir.dt.float32
    f16 = mybir.dt.float16
    f8 = mybir.dt.float8e4
    DR = mybir.MatmulPerfMode.DoubleRow

    x8_in = nc.dram_tensor("x8", [C, IMG], f8, kind="ExternalInput").ap()
    wt_in = nc.dram_tensor("wt", [P, 72 * 256], f8, kind="ExternalInput").ap()
    dg_in = nc.dram_tensor("dg", [P, 2 * 1152], f8, kind="ExternalInput").ap()
    s2_in = nc.dram_tensor("s2", [P, 4 * 512], f32, kind="ExternalInput").ap()
    brow_in = nc.dram_tensor("brow", [P, 512], f32, kind="ExternalInput").ap()
    prm_in = nc.dram_tensor("prm", [P, CT * 11 + CT], f32,
                            kind="ExternalInput").ap()
    out_ext = nc.dram_tensor("out", [C, HW], f32, kind="ExternalOutput").ap()

    with tile.TileContext(nc) as tc:
        with (
            tc.tile_pool(name="wpool", bufs=1) as wpool,
            tc.tile_pool(name="xpool", bufs=2) as xpool,
            tc.tile_pool(name="vpool", bufs=1) as vpool,
            tc.tile_pool(name="accpool", bufs=2) as accpool,
            tc.tile_pool(name="ypool", bufs=4) as ypool,
            tc.tile_pool(name="smpool", bufs=8) as smpool,
            tc.tile_pool(name="prmpool", bufs=4) as prmpool,
            tc.tile_pool(name="opool", bufs=3) as opool,
            tc.tile_pool(name="psum", bufs=8, space="PSUM") as psum,
        ):
            # ---- input DMAs over the three available rings (SP, ACT,
            # POOL), ordered by when each tensor is first needed.
            # scalar ring: dw diag weights + params + first half of the
            # dense weights (co-major blocks: co0/co1 land first).
            dg_sb = wpool.tile([P, 2 * 1152], f8, name="dg_sb", tag="dg")
            nc.scalar.dma_start(dg_sb[:], dg_in[:])
            prm_all = prmpool.tile([P, CT * 11 + CT], f32, name="prm_all",
                                   tag="prm")
            nc.scalar.dma_start(prm_all[:], prm_in[:])
            wt_sb = wpool.tile([P, 72 * 256], f8, name="wt_sb", tag="wt")
            nc.scalar.dma_start(wt_sb[:, 0:36 * 256], wt_in[:, 0:36 * 256])

            # sync ring: fp8 x, PE tiles first (split halves so the diag
            # matmuls start on the first rows), then the DVE tiles, then
            # the rest of the dense weights.
            x8t = {}
            hh8 = (PADH // 2) * IMGC
            for j, t in enumerate(PE_TILES):
                xp = xpool.tile([P, IMG], f8, name=f"x8_{t}", tag=f"x8{t}",
                                bufs=1)
                nc.sync.dma_start(xp[:, 0:hh8], x8_in[t * P:t * P + P, 0:hh8])
                nc.sync.dma_start(xp[:, hh8:IMG],
                                  x8_in[t * P:t * P + P, hh8:IMG])
                x8t[t] = xp.rearrange("p (h w) -> p h w", h=PADH)
            for t in DVE_TILES:
                xp = xpool.tile([P, IMG], f8, name=f"x8_{t}", tag=f"x8{t}",
                                bufs=1)
                nc.sync.dma_start(xp[:], x8_in[t * P:t * P + P, :])
                x8t[t] = xp
            nc.sync.dma_start(wt_sb[:, 36 * 256:72 * 256],
                              wt_in[:, 36 * 256:72 * 256])

            # gpsimd ring: bias-constant operands
            s2_sb = wpool.tile([P, 4 * 512], f32, name="s2_sb", tag="s2")
            nc.gpsimd.dma_start(s2_sb[:], s2_in[:])
            brow_sb = wpool.tile([P, 512], f32, name="brow_sb", tag="brow")
            nc.gpsimd.dma_start(brow_sb[:], brow_in[:])

            prms = [prm_all[:, t * 11:(t + 1) * 11] for t in range(CT)]
            cb_sb = prm_all[:, CT * 11:CT * 11 + CT]

            # upconvert the DVE tiles' x to fp16 on gpsimd (pool engine)
            x16 = {}
            for t in DVE_TILES:
                xu = xpool.tile([P, IMG], f16, name=f"x16_{t}", tag=f"x16{t}",
                                bufs=1)
                nc.gpsimd.tensor_copy(xu[:], x8t[t][:])
                x16[t] = xu.rearrange("p (h w) -> p h w", h=PADH)

            # gpsimd scaled copies for the dx==1 ladder taps (2-byte
            # misaligned on DVE, full speed on pool)
            gys = {}
            for t in DVE_TILES:
                x3 = x16[t]
                prm = prms[t]
                for tap, dy, dx in _taps():
                    if tap in GP_TAPS:
                        y = ypool.tile([P, HW], f16, name="gy", tag="gy",
                                       bufs=4)
                        nc.gpsimd.tensor_scalar_mul(
                            y.rearrange("p (h w) -> p h w", h=H),
                            x3[:, dy:dy + H, dx:dx + W], prm[:, tap:tap + 1])
                        gys[(t, tap)] = y

            # the four v images (fp8, 2^9-scaled varying part of mid) in one
            # tile so the dense DoubleRow pair dim can stride between them.
            v4 = vpool.tile([P, CT, PADH, IMGC], f8, name="v4", tag="v4")

            def stats(t, xin, scratch3, on_dve=False):
                """mean/var of the tile -> per-channel affine (a, tb2) with
                v = a*acc + tb2 (the 2^k scale is folded into prm's wp).
                scratch3 receives the squares (overwritten later)."""
                prm = prms[t]
                sqs = smpool.tile([P, 1], f32, name="sqs", tag="sm")
                ms = smpool.tile([P, 1], f32, name="ms", tag="sm")
                sscr = ypool.tile([P, HW], f16, name="y", tag="y")
                sscr3 = sscr.rearrange("p (h w) -> p h w", h=H)
                if on_dve:
                    nc.vector.scalar_tensor_tensor(
                        scratch3, xin, 1.0, xin,
                        mybir.AluOpType.mult, mybir.AluOpType.mult,
                        accum_out=sqs[:],
                    )
                    nc.vector.scalar_tensor_tensor(
                        sscr3, xin, 0.0, xin,
                        mybir.AluOpType.mult, mybir.AluOpType.add,
                        accum_out=ms[:],
                    )
                else:
                    nc.scalar.activation(
                        scratch3, xin,
                        mybir.ActivationFunctionType.Square, accum_out=sqs[:],
                    )
                    nc.scalar.activation(
                        sscr3, xin,
                        mybir.ActivationFunctionType.Identity, accum_out=ms[:],
                    )
                mu = smpool.tile([P, 1], f32, name="mu", tag="sm")
                nc.vector.tensor_scalar_mul(mu[:], ms[:], 1.0 / HW)
                ex2 = smpool.tile([P, 1], f32, name="ex2", tag="sm")
                nc.vector.tensor_scalar_mul(ex2[:], sqs[:], 1.0 / HW)
                mu2 = smpool.tile([P, 1], f32, name="mu2", tag="sm")
                nc.vector.tensor_mul(mu2[:], mu[:], mu[:])
                ve = smpool.tile([P, 1], f32, name="ve", tag="sm")
                nc.vector.scalar_tensor_tensor(
                    ve[:], mu2[:], -1.0, ex2[:],
                    mybir.AluOpType.mult, mybir.AluOpType.add,
                )
                nc.vector.tensor_scalar_add(ve[:], ve[:], EPS)
                sd = smpool.tile([P, 1], f32, name="sd", tag="sm")
                nc.scalar.sqrt(sd[:], ve[:])
                r = smpool.tile([P, 1], f32, name="r", tag="sm")
                nc.vector.reciprocal(r[:], sd[:])
                a = smpool.tile([P, 1], f32, name="a", tag="a")
                nc.vector.tensor_mul(a[:], r[:], prm[:, 9:10])
                s9 = smpool.tile([P, 1], f32, name="s9", tag="sm")
                nc.vector.tensor_reduce(
                    s9[:], prm[:, 0:9], mybir.AxisListType.X,
                    mybir.AluOpType.add,
                )
                am = smpool.tile([P, 1], f32, name="am", tag="sm")
                nc.vector.tensor_mul(am[:], a[:], mu[:])
                tb = smpool.tile([P, 1], f32, name="tb", tag="tb")
                nc.vector.scalar_tensor_tensor(
                    tb[:], am[:], -1.0, s9[:],
                    mybir.AluOpType.mult, mybir.AluOpType.mult,
                )
                return a, tb

            # ---- depthwise on PE via diagonal fp8 weights; 4 DoubleRow
            # tap-pairs + 1 single matmul per chunk -----------------------
            for j, t in enumerate(PE_TILES):
                x3 = x8t[t]
                scr = accpool.tile([P, HW], f16, name="acc", tag="acc")
                with tc.high_priority():
                    a, tb = stats(t, x3[:, 1:H + 1, 1:W + 1],
                                  scr.rearrange("p (h w) -> p h w", h=H))
                banks = [
                    psum.tile([P, 512], f32, name="bank", tag="bank")
                    for _ in range(NCHUNK)
                ]
                for bi, blk in enumerate(DW_BLOCKS):
                    first, last = bi == 0, bi == len(DW_BLOCKS) - 1
                    if len(blk) == 2:
                        (dyA, dxA), (dyB, dxB) = blk
                        lhsT = dg_sb[:, j * 1152 + bi * 256:
                                     j * 1152 + (bi + 1) * 256].rearrange(
                            "p (two m) -> p two m", two=2)
                        stride = (dyB - dyA) * IMGC + (dxB - dxA)
                        for ch in range(NCHUNK):
                            base = x3[:, ch * 8 + dyA:ch * 8 + dyA + 8,
                                      dxA:dxA + W]
                            rhs = AP(base.tensor, base.offset,
                                     [list(base.ap[0]), [stride, 2],
                                      [IMGC, 8], [1, W]])
                            nc.tensor.matmul(banks[ch][:], lhsT, rhs,
                                             start=first, stop=last,
                                             perf_mode=DR)
                    else:
                        (dy, dx), = blk
                        lhsT = dg_sb[:, j * 1152 + 1024:j * 1152 + 1152]
                        for ch in range(NCHUNK):
                            rhs = x3[:, ch * 8 + dy:ch * 8 + dy + 8, dx:dx + W]
                            nc.tensor.matmul(banks[ch][:], lhsT, rhs,
                                             start=first, stop=last)
                for ch in range(NCHUNK):
                    nc.scalar.activation(
                        v4[:, t, 1 + ch * 8:1 + ch * 8 + 8, 1:W + 1],
                        banks[ch].rearrange("p (h w) -> p h w", h=8),
                        mybir.ActivationFunctionType.Identity,
                        bias=tb[:], scale=a[:],
                    )
                _reflect_borders(nc, v4[:, t])

            # bias constant, exact fp32 on the vector engine:
            # A[co] = sum_ci S[co,ci]*b[ci];  AB[:,co_t] = A + conv_b
            AB = prmpool.tile([P, CT], f32, name="AB", tag="ab")
            ascr = ypool.tile([P, 512], f32, name="ascr", tag="ascr", bufs=1)
            for co_t in range(CT):
                at = smpool.tile([P, 1], f32, name="at", tag="at")
                nc.vector.scalar_tensor_tensor(
                    ascr[:], s2_sb[:, co_t * 512:(co_t + 1) * 512], 1.0,
                    brow_sb[:],
                    mybir.AluOpType.mult, mybir.AluOpType.mult,
                    accum_out=at[:],
                )
                nc.vector.tensor_add(AB[:, co_t:co_t + 1], at[:],
                                     cb_sb[:, co_t:co_t + 1])

            # ---- depthwise on DVE: tensor_scalar (4x) + tensor_add (2x)
            # ladder; dx==1 taps come from the gpsimd copies above --------
            for t in DVE_TILES:
                x3 = x16[t]
                acc = accpool.tile([P, HW], f16, name="acc", tag="acc")
                av = acc.rearrange("p (h w) -> p h w", h=H)
                prm = prms[t]
                sqscr = ypool.tile([P, HW], f16, name="y", tag="y")
                a, tb = stats(t, x3[:, 1:H + 1, 1:W + 1],
                              sqscr.rearrange("p (h w) -> p h w", h=H))
                nc.vector.tensor_scalar_mul(av[:], x3[:, 0:H, 0:W],
                                            prm[:, 0:1])
                for tap, dy, dx in _taps():
                    if tap == 0:
                        continue
                    if tap in GP_TAPS:
                        y = gys[(t, tap)]
                    else:
                        y = ypool.tile([P, HW], f16, name="y", tag="y")
                        nc.vector.tensor_scalar_mul(
                            y.rearrange("p (h w) -> p h w", h=H),
                            x3[:, dy:dy + H, dx:dx + W], prm[:, tap:tap + 1])
                    nc.vector.tensor_add(acc[:], acc[:], y[:])
                nc.scalar.activation(
                    v4[:, t, 1:H + 1, 1:W + 1], av[:],
                    mybir.ActivationFunctionType.Identity,
                    bias=tb[:], scale=a[:],
                )
                _reflect_borders(nc, v4[:, t])

            # ---- dense 3x3: fp8 DoubleRow, pair dim = two ci tiles ------
            out_rr = (nc.sync, nc.scalar, nc.gpsimd)
            n_out = 0
            for co in range(CT):
                banks = [
                    psum.tile([P, 512], f32, name="bank", tag="bank")
                    for _ in range(NCHUNK)
                ]
                for ji, pi in enumerate(PAIR_ORDER):
                    for tap, dy, dx in _taps():
                        idx = (co * 2 + ji) * 9 + tap
                        w_view = wt_sb[:, idx * 256:(idx + 1) * 256].rearrange(
                            "p (two m) -> p two m", two=2)
                        for ch in range(NCHUNK):
                            rhs = v4[:, 2 * pi:2 * pi + 2,
                                     ch * 8 + dy:ch * 8 + dy + 8, dx:dx + W]
                            nc.tensor.matmul(
                                banks[ch][:], w_view, rhs,
                                start=(ji == 0 and tap == 0),
                                stop=(ji == 1 and tap == 8),
                                perf_mode=DR,
                            )
                for ch in range(NCHUNK):
                    o = opool.tile([P, 512], f32, name="o", tag="o")
                    nc.scalar.activation(
                        o[:], banks[ch][:],
                        mybir.ActivationFunctionType.Identity,
                        bias=AB[:, co:co + 1], scale=OUT_SCALE,
                    )
                    out_rr[n_out % 3].dma_start(
                        out_ext[co * P:(co + 1) * P,
                                ch * 512:(ch + 1) * 512],
                        o[:],
                    )
                    n_out += 1

    nc.compile()
    _dedup_ldweights(nc)
    return nc


def kernel(x, w_spatial, w_pointwise, bias, conv_w, conv_b):
    global LAST_EXEC_NS
    if "nc" not in _CACHE:
        _CACHE["nc"] = _build()
    nc = _CACHE["nc"]

    xf = np.asarray(x, dtype=np.float32).astype(np.float16)
    xpad = np.pad(xf, ((0, 0), (0, 0), (1, 1), (1, 1)), mode="reflect")
    ws = np.asarray(w_spatial, dtype=np.float32).reshape(B, C, 9)
    wp = np.asarray(w_pointwise, dtype=np.float32).reshape(B, C)
    bi = np.asarray(bias, dtype=np.float32).reshape(B, C)
    cw = np.asarray(conv_w, dtype=np.float32)
    cb = np.asarray(conv_b, dtype=np.float32)

    # shared dense weights, fp8, emission-order blocks [p, ko, m]:
    # wt[p, ((co*2+j)*9+tap)*256 + ko*128 + m]
    #   = fp8(conv_w[co*128+m, ci*128+p, tap] * 2^M_SC), ci = CI_PAIRS[pi][ko]
    w8 = (cw.reshape(C, C, 9) * (2.0 ** M_SC)).astype(E4)
    wt = np.zeros((P, 72 * 256), dtype=E4)
    w8v = w8.view(np.uint8)
    wtv = wt.view(np.uint8)
    for co in range(CT):
        for ji, pi in enumerate(PAIR_ORDER):
            for tap in range(9):
                idx = (co * 2 + ji) * 9 + tap
                for ko in range(2):
                    ci_t = CI_PAIRS[pi][ko]
                    blk = w8v[co * P:(co + 1) * P,
                              ci_t * P:(ci_t + 1) * P, tap].T
                    wtv[:, idx * 256 + ko * P:idx * 256 + (ko + 1) * P] = blk

    # bias-constant operands: s2[p, co_t*512 + ci] = S[co_t*128+p, ci]
    S = cw.sum(axis=(2, 3))                      # [co, ci]
    s2 = np.zeros((P, 4 * 512), dtype=np.float32)
    for co_t in range(CT):
        s2[:, co_t * 512:(co_t + 1) * 512] = S[co_t * P:(co_t + 1) * P, :]
    cbl = np.ascontiguousarray(cb.reshape(CT, P).T)  # [p, CT]

    in_maps = []
    for b in range(B):
        # fp8 padded x, rows stored at stride 72
        x8 = np.zeros((C, PADH, IMGC), dtype=E4)
        x8[:, :, 0:PADW] = xpad[b].astype(E4)
        # per-channel params; PE tiles use the fp8-rounded taps (scaled
        # 2^S_SC) so the mu*s9 correction matches the diag weights exactly
        prm = np.zeros((CT, P, 11), dtype=np.float32)
        prm[:, :, 0:9] = ws[b].reshape(CT, P, 9)
        prm[:, :, 9] = wp[b].reshape(CT, P)
        for t in PE_TILES:
            wsq = (ws[b, t * P:(t + 1) * P] * (2.0 ** S_SC)).astype(E4)
            prm[t, :, 0:9] = wsq.astype(np.float32)
        for t in DVE_TILES:
            prm[t, :, 9] *= 2.0 ** K_SC
        prm = np.concatenate(
            [prm.transpose(1, 0, 2).reshape(P, CT * 11), cbl], axis=1)
        prm = np.ascontiguousarray(prm)

        # diagonal fp8 depthwise weights for the PE tiles
        dg = np.zeros((P, 2 * 1152), dtype=E4)
        idxp = np.arange(P)
        for j, t in enumerate(PE_TILES):
            wsq = (ws[b, t * P:(t + 1) * P] * (2.0 ** S_SC)).astype(E4)
            for bi2, blk in enumerate(DW_BLOCKS):
                if len(blk) == 2:
                    for ko, (dy, dx) in enumerate(blk):
                        dg[idxp, j * 1152 + bi2 * 256 + ko * P + idxp] = \
                            wsq[:, dy * 3 + dx]
                else:
                    (dy, dx), = blk
                    dg[idxp, j * 1152 + 1024 + idxp] = wsq[:, dy * 3 + dx]

        brow = np.ascontiguousarray(
            np.broadcast_to(bi[b][None, :], (P, C)).astype(np.float32))

        in_maps.append({
            "x8": x8.reshape(C, IMG),
            "wt": wt,
            "dg": dg,
            "s2": s2,
            "brow": brow,
            "prm": prm,
        })

    res = run_bass_kernel_spmd(
        nc, in_maps, list(range(N_CORES)), trace=_TRACE
    )
    LAST_EXEC_NS = res.exec_time_ns
    out = np.stack([res.results[b]["out"].reshape(C, H, W) for b in range(B)])
    return out


# revision 7
# speedup vs baseline: 3.4025x; 3.4025x over previous
"""Trainium2 Bass kernel for AdaConv2d (instance-norm + per-sample dynamic
depthwise 3x3 conv + per-channel scale/bias + shared dense 3x3 conv, reflect
padding everywhere).

Data-parallel over batch: 8 samples -> 8 NeuronCores, one sample per core.

Math (per sample, per channel c):
    xn   = (x - mu_c) * rsqrt(var_c + eps)
    mid  = wp_c * depthwise3x3(reflect_pad(xn); ws_c) + b_c
         = v_c + b_c    with v = a*dw(x) - a*mu*s9,  a = wp*rsqrt(var+eps)
    out  = dense3x3(reflect_pad(mid); conv_w) + conv_b
         = dense3x3(reflect_pad(v); conv_w) + S@b + conv_b
      where S[co,ci] = sum_taps conv_w  (reflect padding makes the per-channel
      constant b contribute exactly S@b at every output pixel).

The v split is what makes fp8 viable: v has std ~0.0075 (vs mid's ~0.05
dominated by the per-channel constant b), so quantizing v*2^9 to fp8e4m3
keeps the dense-conv error ~1e-2 relative (gate is 2e-2).  The dense conv
runs as fp8 DoubleRow matmuls: each matmul contracts 256 = 2 channel tiles
(the pair dim of the moving AP strides between two v images that live in
one SBUF tile), at ~1.7x the bf16 rate.  The constant S@b + conv_b is
computed exactly in fp32 on the vector engine (elementwise mult with a
host-broadcast b row + row-reduce) and added as the eviction bias.

All four depthwise tiles run on the tensor engine as diagonal-fp8-weight
matmuls, DoubleRow-pairing two taps per matmul (pair dim = the 2-byte or
2-row shift between the taps' windows).  The vector engine only does the
psum evictions (fused scale+bias tensor_scalar), stats small-ops, reflect
borders and the bias constant; the scalar engine runs the four stats
passes.  PSUM cycles as 4-bank [128,2048] group tiles so the scheduler
keeps each weight block's 8 chunk matmuls contiguous (one LDWEIGHTS per
block instead of per matmul).
"""

import os
import sys
import types

import numpy as np
import ml_dtypes

B, C, H, W = 8, 512, 64, 64
KS = 3
EPS = 1e-5
N_CORES = 8
P = 128
CT = C // P            # 4 channel tiles
PADH = H + 2           # 66
PADW = W + 2           # 66
IMGC = 72              # stored row stride (16B-aligned images: 66*72 = 4752)
IMG = PADH * IMGC      # 4752
HW = H * W             # 4096
NCHUNK = 8             # 8-row output chunks -> one psum bank each
TILE_ORDER = (2, 3, 0, 1)  # depthwise production order (dense eats 2,3 first)
CI_PAIRS = ((0, 1), (2, 3))
PAIR_ORDER = (1, 0)    # dense contraction: pair (2,3) first (ready early)
K_SC = 9               # v scale 2^9
M_SC = 9               # dense weight scale 2^9
S_SC = 9               # depthwise diag weight scale 2^9
OUT_SCALE = float(2.0 ** (-(K_SC + M_SC)))
# depthwise tap blocks: 4 DoubleRow pairs + 1 single
DW_BLOCKS = (((0, 0), (0, 2)), ((1, 0), (1, 2)), ((2, 0), (2, 2)),
             ((0, 1), (2, 1)), ((1, 1),))

E4 = ml_dtypes.float8_e4m3


def _install_ntff_hook():
    """Register the NTFF profiling hook that concourse expects under axon
    (missing antenv.axon_hooks module in this image)."""
    if "antenv.axon_hooks" in sys.modules:
        return
    try:
        mod = types.ModuleType("antenv.axon_hooks")
        holder = [None]
        mod.set_axon_ntff_profile_hook = lambda h: holder.__setitem__(0, h)
        mod.get_axon_ntff_profile_hook = lambda: holder[0]
        sys.modules["antenv.axon_hooks"] = mod
        from trn_agent_boot.trn_boot import _ntff_profile_via_ctypes

        hook = _ntff_profile_via_ctypes("/opt/axon/libaxon_pjrt.so")
        mod.set_axon_ntff_profile_hook(hook)
    except Exception:
        sys.modules.pop("antenv.axon_hooks", None)


_TRACE = os.environ.get("BASS_KERNEL_TRACE") == "1"
if _TRACE:
    _install_ntff_hook()

import concourse.tile as tile
from concourse import bacc, mybir
import concourse.bass_utils as bass_utils
from concourse.bass_utils import run_bass_kernel_spmd
from concourse.ap import AP

if _TRACE:
    bass_utils.upload_artifacts = lambda d: d

LAST_EXEC_NS = None
_CACHE = {}


def _taps():
    for tap in range(KS * KS):
        yield tap, tap // KS, tap % KS


def _reflect_borders(nc, img3):
    """Fill the 1-wide reflect border of a [128, 66, 72] image whose
    interior [1:65, 1:65] is already populated (cols first, then full rows
    so the corners come out as reflect-of-reflect, matching np.pad)."""
    nc.vector.tensor_copy(img3[:, 1:H + 1, 0:1], img3[:, 1:H + 1, 2:3])
    nc.vector.tensor_copy(img3[:, 1:H + 1, PADW - 1:PADW],
                          img3[:, 1:H + 1, PADW - 3:PADW - 2])
    nc.vector.tensor_copy(img3[:, 0:1, 0:PADW], img3[:, 2:3, 0:PADW])
    nc.vector.tensor_copy(img3[:, PADH - 1:PADH, 0:PADW],
                          img3[:, PADH - 3:PADH - 2, 0:PADW])


def _dedup_ldweights(nc):
    """Drop InstLdweights whose weights AP is identical to the previous
    weight load on the PE stream (bacc splits every matmul into LDW+MM;
    with one weight block reused across 8 chunk matmuls, 7 of 8 loads are
    redundant and serialize with the matmuls).  LDWs carrying semaphore
    waits/updates are kept."""
    n_removed = 0
    for f in nc.m.functions:
        for bb in f.blocks:
            insts = bb.instructions
            keep = []
            last_key = None
            for inst in insts:
                tn = type(inst).__name__
                if tn == "InstLdweights":
                    si = inst.sync_info
                    has_sync = si is not None and (
                        len(si.on_wait) > 0 or len(si.on_update) > 0
                    )
                    key = repr(inst.ins[0])
                    if key == last_key and not has_sync:
                        n_removed += 1
                        continue
                    last_key = key
                elif tn == "InstMatmult":
                    if getattr(inst, "is_transpose", False):
                        last_key = None
                keep.append(inst)
            if len(keep) != len(insts):
                bb.instructions = keep
    return n_removed


def _build():
    nc = bacc.Bacc("TRN2", target_bir_lowering=False, debug=False,
                   num_devices=N_CORES)
    f32 = mybir.dt.float32
    f16 = mybir.dt.float16
    f8 = mybir.dt.float8e4
    DR = mybir.MatmulPerfMode.DoubleRow

    x8_in = nc.dram_tensor("x8", [C, IMG], f8, kind="ExternalInput").ap()
    wt_in = nc.dram_tensor("wt", [P, 72 * 256], f8, kind="ExternalInput").ap()
    dg_in = nc.dram_tensor("dg", [P, 4 * 1152], f8, kind="ExternalInput").ap()
    s2_in = nc.dram_tensor("s2", [P, 4 * 512], f32, kind="ExternalInput").ap()
    brow_in = nc.dram_tensor("brow", [P, 512], f32, kind="ExternalInput").ap()
    prm_in = nc.dram_tensor("prm", [P, CT * 11 + CT], f32,
                            kind="ExternalInput").ap()
    out_ext = nc.dram_tensor("out", [C, HW], f32, kind="ExternalOutput").ap()

    with tile.TileContext(nc) as tc:
        with (
            tc.tile_pool(name="wpool", bufs=1) as wpool,
            tc.tile_pool(name="xpool", bufs=2) as xpool,
            tc.tile_pool(name="vpool", bufs=1) as vpool,
            tc.tile_pool(name="accpool", bufs=2) as accpool,
            tc.tile_pool(name="ypool", bufs=4) as ypool,
            tc.tile_pool(name="smpool", bufs=8) as smpool,
            tc.tile_pool(name="prmpool", bufs=4) as prmpool,
            tc.tile_pool(name="opool", bufs=3) as opool,
            tc.tile_pool(name="psum", bufs=2, space="PSUM") as psum,
        ):
            # ---- input DMAs over the three rings (SP, ACT, POOL) --------
            # scalar ring: dw diag weights + params + first half of the
            # dense weights (co-major blocks: co0/co1 land first).
            dg_sb = wpool.tile([P, 4 * 1152], f8, name="dg_sb", tag="dg")
            nc.scalar.dma_start(dg_sb[:], dg_in[:])
            prm_all = prmpool.tile([P, CT * 11 + CT], f32, name="prm_all",
                                   tag="prm")
            nc.scalar.dma_start(prm_all[:], prm_in[:])
            wt_sb = wpool.tile([P, 72 * 256], f8, name="wt_sb", tag="wt")
            nc.scalar.dma_start(wt_sb[:, 0:36 * 256], wt_in[:, 0:36 * 256])

            # sync ring: fp8 x in production order, split halves aligned to
            # the 4-chunk psum groups so group-0 matmuls start early.
            x8t = {}
            hh8 = (PADH // 2) * IMGC
            for t in TILE_ORDER:
                xp = xpool.tile([P, IMG], f8, name=f"x8_{t}", tag=f"x8{t}",
                                bufs=1)
                nc.sync.dma_start(xp[:, 0:hh8], x8_in[t * P:t * P + P, 0:hh8])
                nc.sync.dma_start(xp[:, hh8:IMG],
                                  x8_in[t * P:t * P + P, hh8:IMG])
                x8t[t] = xp.rearrange("p (h w) -> p h w", h=PADH)
            nc.sync.dma_start(wt_sb[:, 36 * 256:72 * 256],
                              wt_in[:, 36 * 256:72 * 256])

            # gpsimd ring: bias-constant operands
            s2_sb = wpool.tile([P, 4 * 512], f32, name="s2_sb", tag="s2")
            nc.gpsimd.dma_start(s2_sb[:], s2_in[:])
            brow_sb = wpool.tile([P, 512], f32, name="brow_sb", tag="brow")
            nc.gpsimd.dma_start(brow_sb[:], brow_in[:])

            prms = [prm_all[:, t * 11:(t + 1) * 11] for t in range(CT)]
            cb_sb = prm_all[:, CT * 11:CT * 11 + CT]

            # the four v images (fp8, 2^9-scaled varying part of mid) in one
            # tile so the dense DoubleRow pair dim can stride between them.
            v4 = vpool.tile([P, CT, PADH, IMGC], f8, name="v4", tag="v4")

            def stats(t, xin, scratch3):
                """mean/var of the tile -> per-channel affine (a, tb2) with
                v = a*acc + tb2 (the 2^k scale is folded into prm's wp).
                scratch3 receives the squares (overwritten later)."""
                prm = prms[t]
                sqs = smpool.tile([P, 1], f32, name="sqs", tag="sm")
                ms = smpool.tile([P, 1], f32, name="ms", tag="sm")
                sscr = ypool.tile([P, HW], f16, name="y", tag="y")
                sscr3 = sscr.rearrange("p (h w) -> p h w", h=H)
                nc.scalar.activation(
                    scratch3, xin,
                    mybir.ActivationFunctionType.Square, accum_out=sqs[:],
                )
                nc.scalar.activation(
                    sscr3, xin,
                    mybir.ActivationFunctionType.Identity, accum_out=ms[:],
                )
                mu = smpool.tile([P, 1], f32, name="mu", tag="sm")
                nc.vector.tensor_scalar_mul(mu[:], ms[:], 1.0 / HW)
                ex2 = smpool.tile([P, 1], f32, name="ex2", tag="sm")
                nc.vector.tensor_scalar_mul(ex2[:], sqs[:], 1.0 / HW)
                mu2 = smpool.tile([P, 1], f32, name="mu2", tag="sm")
                nc.vector.tensor_mul(mu2[:], mu[:], mu[:])
                ve = smpool.tile([P, 1], f32, name="ve", tag="sm")
                nc.vector.scalar_tensor_tensor(
                    ve[:], mu2[:], -1.0, ex2[:],
                    mybir.AluOpType.mult, mybir.AluOpType.add,
                )
                nc.vector.tensor_scalar_add(ve[:], ve[:], EPS)
                sd = smpool.tile([P, 1], f32, name="sd", tag="sm")
                nc.scalar.sqrt(sd[:], ve[:])
                r = smpool.tile([P, 1], f32, name="r", tag="sm")
                nc.vector.reciprocal(r[:], sd[:])
                a = smpool.tile([P, 1], f32, name="a", tag="a")
                nc.vector.tensor_mul(a[:], r[:], prm[:, 9:10])
                s9 = smpool.tile([P, 1], f32, name="s9", tag="sm")
                nc.vector.tensor_reduce(
                    s9[:], prm[:, 0:9], mybir.AxisListType.X,
                    mybir.AluOpType.add,
                )
                am = smpool.tile([P, 1], f32, name="am", tag="sm")
                nc.vector.tensor_mul(am[:], a[:], mu[:])
                tb = smpool.tile([P, 1], f32, name="tb", tag="tb")
                nc.vector.scalar_tensor_tensor(
                    tb[:], am[:], -1.0, s9[:],
                    mybir.AluOpType.mult, mybir.AluOpType.mult,
                )
                return a, tb

            # ---- depthwise on PE via diagonal fp8 weights; 4 DoubleRow
            # tap-pairs + 1 single matmul per chunk.  Evictions run on the
            # vector engine as fused (psum*a + tb) tensor_scalar, one per
            # 4-chunk psum group.
            for j, t in enumerate(TILE_ORDER):
                x3 = x8t[t]
                scr = accpool.tile([P, HW], f16, name="acc", tag="acc")
                with tc.high_priority(offset=None if j < 2 else 0):
                    a, tb = stats(t, x3[:, 1:H + 1, 1:W + 1],
                                  scr.rearrange("p (h w) -> p h w", h=H))
                groups = [
                    psum.tile([P, 2048], f32, name="bank", tag="bank")
                    for _ in range(2)
                ]
                for bi, blk in enumerate(DW_BLOCKS):
                    first, last = bi == 0, bi == len(DW_BLOCKS) - 1
                    if len(blk) == 2:
                        (dyA, dxA), (dyB, dxB) = blk
                        lhsT = dg_sb[:, t * 1152 + bi * 256:
                                     t * 1152 + (bi + 1) * 256].rearrange(
                            "p (two m) -> p two m", two=2)
                        stride = (dyB - dyA) * IMGC + (dxB - dxA)
                        for ch in range(NCHUNK):
                            base = x3[:, ch * 8 + dyA:ch * 8 + dyA + 8,
                                      dxA:dxA + W]
                            rhs = AP(base.tensor, base.offset,
                                     [list(base.ap[0]), [stride, 2],
                                      [IMGC, 8], [1, W]])
                            nc.tensor.matmul(
                                groups[ch // 4][:, (ch % 4) * 512:
                                                (ch % 4) * 512 + 512],
                                lhsT, rhs, start=first, stop=last,
                                perf_mode=DR)
                    else:
                        (dy, dx), = blk
                        lhsT = dg_sb[:, t * 1152 + 1024:t * 1152 + 1152]
                        for ch in range(NCHUNK):
                            rhs = x3[:, ch * 8 + dy:ch * 8 + dy + 8, dx:dx + W]
                            nc.tensor.matmul(
                                groups[ch // 4][:, (ch % 4) * 512:
                                                (ch % 4) * 512 + 512],
                                lhsT, rhs, start=first, stop=last)
                for gi in range(2):
                    nc.vector.tensor_scalar(
                        v4[:, t, 1 + gi * 32:1 + gi * 32 + 32, 1:W + 1],
                        groups[gi].rearrange("p (h w) -> p h w", h=32),
                        a[:], tb[:],
                        mybir.AluOpType.mult, mybir.AluOpType.add,
                    )
                _reflect_borders(nc, v4[:, t])

            # bias constant, exact fp32 on the vector engine:
            # A[co] = sum_ci S[co,ci]*b[ci];  AB[:,co_t] = A + conv_b
            AB = prmpool.tile([P, CT], f32, name="AB", tag="ab")
            ascr = ypool.tile([P, 512], f32, name="ascr", tag="ascr", bufs=1)
            for co_t in range(CT):
                at = smpool.tile([P, 1], f32, name="at", tag="at")
                nc.vector.scalar_tensor_tensor(
                    ascr[:], s2_sb[:, co_t * 512:(co_t + 1) * 512], 1.0,
                    brow_sb[:],
                    mybir.AluOpType.mult, mybir.AluOpType.mult,
                    accum_out=at[:],
                )
                nc.vector.tensor_add(AB[:, co_t:co_t + 1], at[:],
                                     cb_sb[:, co_t:co_t + 1])

            # ---- dense 3x3: fp8 DoubleRow, pair dim = two ci tiles ------
            out_rr = (nc.sync, nc.scalar, nc.gpsimd)
            n_out = 0
            for co in range(CT):
                groups = [
                    psum.tile([P, 2048], f32, name="bank", tag="bank")
                    for _ in range(2)
                ]
                for ji, pi in enumerate(PAIR_ORDER):
                    for tap, dy, dx in _taps():
                        idx = (co * 2 + ji) * 9 + tap
                        w_view = wt_sb[:, idx * 256:(idx + 1) * 256].rearrange(
                            "p (two m) -> p two m", two=2)
                        for ch in range(NCHUNK):
                            rhs = v4[:, 2 * pi:2 * pi + 2,
                                     ch * 8 + dy:ch * 8 + dy + 8, dx:dx + W]
                            nc.tensor.matmul(
                                groups[ch // 4][:, (ch % 4) * 512:
                                                (ch % 4) * 512 + 512],
                                w_view, rhs,
                                start=(ji == 0 and tap == 0),
                                stop=(ji == 1 and tap == 8),
                                perf_mode=DR,
                            )
                for gi in range(2):
                    o = opool.tile([P, 2048], f32, name="o", tag="o")
                    nc.vector.tensor_scalar(
                        o[:], groups[gi][:], OUT_SCALE, AB[:, co:co + 1],
                        mybir.AluOpType.mult, mybir.AluOpType.add,
                    )
                    out_rr[n_out % 3].dma_start(
                        out_ext[co * P:(co + 1) * P,
                                gi * 2048:(gi + 1) * 2048],
                        o[:],
                    )
                    n_out += 1

    nc.compile()
    _dedup_ldweights(nc)
    return nc


def kernel(x, w_spatial, w_pointwise, bias, conv_w, conv_b):
    global LAST_EXEC_NS
    if "nc" not in _CACHE:
        _CACHE["nc"] = _build()
    nc = _CACHE["nc"]

    xf = np.asarray(x, dtype=np.float32).astype(np.float16)
    xpad = np.pad(xf, ((0, 0), (0, 0), (1, 1), (1, 1)), mode="reflect")
    ws = np.asarray(w_spatial, dtype=np.float32).reshape(B, C, 9)
    wp = np.asarray(w_pointwise, dtype=np.float32).reshape(B, C)
    bi = np.asarray(bias, dtype=np.float32).reshape(B, C)
    cw = np.asarray(conv_w, dtype=np.float32)
    cb = np.asarray(conv_b, dtype=np.float32)

    # shared dense weights, fp8, emission-order blocks [p, ko, m]:
    # wt[p, ((co*2+j)*9+tap)*256 + ko*128 + m]
    #   = fp8(conv_w[co*128+m, ci*128+p, tap] * 2^M_SC), ci = CI_PAIRS[pi][ko]
    w8 = (cw.reshape(C, C, 9) * (2.0 ** M_SC)).astype(E4)
    wt = np.zeros((P, 72 * 256), dtype=E4)
    w8v = w8.view(np.uint8)
    wtv = wt.view(np.uint8)
    for co in range(CT):
        for ji, pi in enumerate(PAIR_ORDER):
            for tap in range(9):
                idx = (co * 2 + ji) * 9 + tap
                for ko in range(2):
                    ci_t = CI_PAIRS[pi][ko]
                    blk = w8v[co * P:(co + 1) * P,
                              ci_t * P:(ci_t + 1) * P, tap].T
                    wtv[:, idx * 256 + ko * P:idx * 256 + (ko + 1) * P] = blk

    # bias-constant operands: s2[p, co_t*512 + ci] = S[co_t*128+p, ci]
    S = cw.sum(axis=(2, 3))                      # [co, ci]
    s2 = np.zeros((P, 4 * 512), dtype=np.float32)
    for co_t in range(CT):
        s2[:, co_t * 512:(co_t + 1) * 512] = S[co_t * P:(co_t + 1) * P, :]
    cbl = np.ascontiguousarray(cb.reshape(CT, P).T)  # [p, CT]

    in_maps = []
    for b in range(B):
        # fp8 padded x, rows stored at stride 72
        x8 = np.zeros((C, PADH, IMGC), dtype=E4)
        x8[:, :, 0:PADW] = xpad[b].astype(E4)
        # per-channel params; the fp8-rounded taps (scaled 2^S_SC) go in so
        # the mu*s9 correction matches the diag weights exactly
        wsq8 = (ws[b] * (2.0 ** S_SC)).astype(E4)
        prm = np.zeros((CT, P, 11), dtype=np.float32)
        prm[:, :, 0:9] = wsq8.astype(np.float32).reshape(CT, P, 9)
        prm[:, :, 9] = wp[b].reshape(CT, P)
        prm = np.concatenate(
            [prm.transpose(1, 0, 2).reshape(P, CT * 11), cbl], axis=1)
        prm = np.ascontiguousarray(prm)

        # diagonal fp8 depthwise weights, one 1152-col block per tile
        dg = np.zeros((P, 4 * 1152), dtype=E4)
        idxp = np.arange(P)
        for t in range(CT):
            wsq = wsq8[t * P:(t + 1) * P]
            for bi2, blk in enumerate(DW_BLOCKS):
                if len(blk) == 2:
                    for ko, (dy, dx) in enumerate(blk):
                        dg[idxp, t * 1152 + bi2 * 256 + ko * P + idxp] = \
                            wsq[:, dy * 3 + dx]
                else:
                    (dy, dx), = blk
                    dg[idxp, t * 1152 + 1024 + idxp] = wsq[:, dy * 3 + dx]

        brow = np.ascontiguousarray(
            np.broadcast_to(bi[b][None, :], (P, C)).astype(np.float32))

        in_maps.append({
            "x8": x8.reshape(C, IMG),
            "wt": wt,
            "dg": dg,
            "s2": s2,
            "brow": brow,
            "prm": prm,
        })

    res = run_bass_kernel_spmd(
        nc, in_maps, list(range(N_CORES)), trace=_TRACE
    )
    LAST_EXEC_NS = res.exec_time_ns
    out = np.stack([res.results[b]["out"].reshape(C, H, W) for b in range(B)])
    return out


# revision 12
# speedup vs baseline: 3.5770x; 1.0513x over previous
"""Trainium2 Bass kernel for AdaConv2d (instance-norm + per-sample dynamic
depthwise 3x3 conv + per-channel scale/bias + shared dense 3x3 conv, reflect
padding everywhere).

Data-parallel over batch: 8 samples -> 8 NeuronCores, one sample per core.

Math (per sample, per channel c):
    xn   = (x - mu_c) * rsqrt(var_c + eps)
    mid  = wp_c * depthwise3x3(reflect_pad(xn); ws_c) + b_c
         = v_c + b_c    with v = a*dw(x) - a*mu*s9,  a = wp*rsqrt(var+eps)
    out  = dense3x3(reflect_pad(mid); conv_w) + conv_b
         = dense3x3(reflect_pad(v); conv_w) + S@b + conv_b
      where S[co,ci] = sum_taps conv_w  (reflect padding makes the per-channel
      constant b contribute exactly S@b at every output pixel).

The v split is what makes fp8 viable: v has std ~0.0075 (vs mid's ~0.05
dominated by the per-channel constant b), so quantizing v*2^9 to fp8e4m3
keeps the dense-conv error ~1e-2 relative (gate is 2e-2).  The dense conv
runs as fp8 DoubleRow matmuls: each matmul contracts 256 = 2 channel tiles
(the pair dim of the moving AP strides between two v images that live in
one SBUF tile), at ~1.7x the bf16 rate.  The constant S@b + conv_b is
computed exactly in fp32 on the vector engine (elementwise mult with a
host-broadcast b row + row-reduce) and added as the eviction bias.

All four depthwise tiles run on the tensor engine as diagonal-fp8-weight
matmuls, DoubleRow-pairing two taps per matmul (pair dim = the 2-byte or
2-row shift between the taps' windows).  The vector engine only does the
psum evictions (fused scale+bias tensor_scalar), stats small-ops, reflect
borders and the bias constant; the scalar engine runs the four stats
passes.  PSUM cycles as 4-bank [128,2048] group tiles so the scheduler
keeps each weight block's 8 chunk matmuls contiguous (one LDWEIGHTS per
block instead of per matmul).
"""

import os
import sys
import types

import numpy as np
import ml_dtypes

B, C, H, W = 8, 512, 64, 64
KS = 3
EPS = 1e-5
N_CORES = 8
P = 128
CT = C // P            # 4 channel tiles
PADH = H + 2           # 66
PADW = W + 2           # 66
IMGC = 72              # stored row stride (16B-aligned images: 66*72 = 4752)
IMG = PADH * IMGC      # 4752
HW = H * W             # 4096
NCHUNK = 8             # 8-row output chunks -> one psum bank each
TILE_ORDER = (2, 3, 0, 1)  # depthwise production order (dense eats 2,3 first)
CI_PAIRS = ((0, 1), (2, 3))
PAIR_ORDER = (1, 0)    # dense contraction: pair (2,3) first (ready early)
K_SC = 9               # v scale 2^9
M_SC = 9               # dense weight scale 2^9
S_SC = 9               # depthwise diag weight scale 2^9
OUT_SCALE = float(2.0 ** (-(K_SC + M_SC)))
# depthwise tap blocks: 4 DoubleRow pairs + 1 single
DW_BLOCKS = (((0, 0), (0, 2)), ((1, 0), (1, 2)), ((2, 0), (2, 2)),
             ((0, 1), (2, 1)), ((1, 1),))

E4 = ml_dtypes.float8_e4m3


def _install_ntff_hook():
    """Register the NTFF profiling hook that concourse expects under axon
    (missing antenv.axon_hooks module in this image)."""
    if "antenv.axon_hooks" in sys.modules:
        return
    try:
        mod = types.ModuleType("antenv.axon_hooks")
        holder = [None]
        mod.set_axon_ntff_profile_hook = lambda h: holder.__setitem__(0, h)
        mod.get_axon_ntff_profile_hook = lambda: holder[0]
        sys.modules["antenv.axon_hooks"] = mod
        from trn_agent_boot.trn_boot import _ntff_profile_via_ctypes

        hook = _ntff_profile_via_ctypes("/opt/axon/libaxon_pjrt.so")
        mod.set_axon_ntff_profile_hook(hook)
    except Exception:
        sys.modules.pop("antenv.axon_hooks", None)


_TRACE = os.environ.get("BASS_KERNEL_TRACE") == "1"
if _TRACE:
    _install_ntff_hook()

import concourse.tile as tile
from concourse import bacc, mybir
import concourse.bass_utils as bass_utils
from concourse.bass_utils import run_bass_kernel_spmd
from concourse.ap import AP

if _TRACE:
    bass_utils.upload_artifacts = lambda d: d

LAST_EXEC_NS = None
_CACHE = {}


def _taps():
    for tap in range(KS * KS):
        yield tap, tap // KS, tap % KS


def _reflect_borders(nc, img3):
    """Fill the 1-wide reflect border of a [128, 66, 72] image whose
    interior [1:65, 1:65] is already populated (cols first, then full rows
    so the corners come out as reflect-of-reflect, matching np.pad)."""
    nc.vector.tensor_copy(img3[:, 1:H + 1, 0:1], img3[:, 1:H + 1, 2:3])
    nc.vector.tensor_copy(img3[:, 1:H + 1, PADW - 1:PADW],
                          img3[:, 1:H + 1, PADW - 3:PADW - 2])
    nc.vector.tensor_copy(img3[:, 0:1, 0:PADW], img3[:, 2:3, 0:PADW])
    nc.vector.tensor_copy(img3[:, PADH - 1:PADH, 0:PADW],
                          img3[:, PADH - 3:PADH - 2, 0:PADW])


def _dedup_ldweights(nc):
    """Drop InstLdweights whose weights AP is identical to the previous
    weight load on the PE stream (bacc splits every matmul into LDW+MM;
    with one weight block reused across 8 chunk matmuls, 7 of 8 loads are
    redundant and serialize with the matmuls).  LDWs carrying semaphore
    waits/updates are kept."""
    n_removed = 0
    for f in nc.m.functions:
        for bb in f.blocks:
            insts = bb.instructions
            keep = []
            last_key = None
            for inst in insts:
                tn = type(inst).__name__
                if tn == "InstLdweights":
                    si = inst.sync_info
                    has_sync = si is not None and (
                        len(si.on_wait) > 0 or len(si.on_update) > 0
                    )
                    key = repr(inst.ins[0])
                    if key == last_key and not has_sync:
                        n_removed += 1
                        continue
                    last_key = key
                elif tn == "InstMatmult":
                    if getattr(inst, "is_transpose", False):
                        last_key = None
                keep.append(inst)
            if len(keep) != len(insts):
                bb.instructions = keep
    return n_removed


def _build():
    nc = bacc.Bacc("TRN2", target_bir_lowering=False, debug=False,
                   num_devices=N_CORES)
    f32 = mybir.dt.float32
    f16 = mybir.dt.float16
    f8 = mybir.dt.float8e4
    DR = mybir.MatmulPerfMode.DoubleRow

    x8_in = nc.dram_tensor("x8", [C, IMG], f8, kind="ExternalInput").ap()
    wt_in = nc.dram_tensor("wt", [P, 72 * 256], f8, kind="ExternalInput").ap()
    dg_in = nc.dram_tensor("dg", [P, 4 * 1152], f8, kind="ExternalInput").ap()
    s2_in = nc.dram_tensor("s2", [P, 4 * 512], f32, kind="ExternalInput").ap()
    brow_in = nc.dram_tensor("brow", [P, 512], f32, kind="ExternalInput").ap()
    prm_in = nc.dram_tensor("prm", [P, CT * 11 + CT], f32,
                            kind="ExternalInput").ap()
    out_ext = nc.dram_tensor("out", [C, HW], f32, kind="ExternalOutput").ap()

    with tile.TileContext(nc) as tc:
        with (
            tc.tile_pool(name="wpool", bufs=1) as wpool,
            tc.tile_pool(name="xpool", bufs=2) as xpool,
            tc.tile_pool(name="vpool", bufs=1) as vpool,
            tc.tile_pool(name="accpool", bufs=2) as accpool,
            tc.tile_pool(name="ypool", bufs=4) as ypool,
            tc.tile_pool(name="smpool", bufs=8) as smpool,
            tc.tile_pool(name="prmpool", bufs=4) as prmpool,
            tc.tile_pool(name="opool", bufs=4) as opool,
            tc.tile_pool(name="psum", bufs=4, space="PSUM") as psum,
        ):
            # ---- input DMAs over the three rings (SP, ACT, POOL) --------
            # scalar ring: dw diag weights + params + first half of the
            # dense weights (co-major blocks: co0/co1 land first).
            dg_sb = wpool.tile([P, 4 * 1152], f8, name="dg_sb", tag="dg")
            for t in TILE_ORDER:
                nc.scalar.dma_start(dg_sb[:, t * 1152:(t + 1) * 1152],
                                    dg_in[:, t * 1152:(t + 1) * 1152])
            prm_all = prmpool.tile([P, CT * 11 + CT], f32, name="prm_all",
                                   tag="prm")
            nc.scalar.dma_start(prm_all[:], prm_in[:])
            wt_sb = wpool.tile([P, 72 * 256], f8, name="wt_sb", tag="wt")
            nc.scalar.dma_start(wt_sb[:, 0:36 * 256], wt_in[:, 0:36 * 256])

            # sync ring: fp8 x in production order, split halves aligned to
            # the 4-chunk psum groups so group-0 matmuls start early.
            x8t = {}
            hh8 = (PADH // 2) * IMGC
            for t in TILE_ORDER:
                xp = xpool.tile([P, IMG], f8, name=f"x8_{t}", tag=f"x8{t}",
                                bufs=1)
                nc.sync.dma_start(xp[:, 0:hh8], x8_in[t * P:t * P + P, 0:hh8])
                nc.sync.dma_start(xp[:, hh8:IMG],
                                  x8_in[t * P:t * P + P, hh8:IMG])
                x8t[t] = xp.rearrange("p (h w) -> p h w", h=PADH)
            nc.sync.dma_start(wt_sb[:, 36 * 256:72 * 256],
                              wt_in[:, 36 * 256:72 * 256])

            # gpsimd ring: bias-constant operands
            s2_sb = wpool.tile([P, 4 * 512], f32, name="s2_sb", tag="s2")
            nc.gpsimd.dma_start(s2_sb[:], s2_in[:])
            brow_sb = wpool.tile([P, 512], f32, name="brow_sb", tag="brow")
            nc.gpsimd.dma_start(brow_sb[:], brow_in[:])

            prms = [prm_all[:, t * 11:(t + 1) * 11] for t in range(CT)]
            cb_sb = prm_all[:, CT * 11:CT * 11 + CT]

            # the four v images (fp8, 2^9-scaled varying part of mid) in one
            # tile so the dense DoubleRow pair dim can stride between them.
            v4 = vpool.tile([P, CT, PADH, IMGC], f8, name="v4", tag="v4")

            def stats(t, xin, scratch3):
                """mean/var of the tile -> per-channel affine (a, tb2) with
                v = a*acc + tb2 (the 2^k scale is folded into prm's wp).
                scratch3 receives the squares (overwritten later)."""
                prm = prms[t]
                sqs = smpool.tile([P, 1], f32, name="sqs", tag="sm")
                ms = smpool.tile([P, 1], f32, name="ms", tag="sm")
                sscr = ypool.tile([P, HW], f16, name="y", tag="y")
                sscr3 = sscr.rearrange("p (h w) -> p h w", h=H)
                nc.scalar.activation(
                    scratch3, xin,
                    mybir.ActivationFunctionType.Square, accum_out=sqs[:],
                )
                nc.scalar.activation(
                    sscr3, xin,
                    mybir.ActivationFunctionType.Identity, accum_out=ms[:],
                )
                mu = smpool.tile([P, 1], f32, name="mu", tag="sm")
                nc.vector.tensor_scalar_mul(mu[:], ms[:], 1.0 / HW)
                ex2 = smpool.tile([P, 1], f32, name="ex2", tag="sm")
                nc.vector.tensor_scalar_mul(ex2[:], sqs[:], 1.0 / HW)
                mu2 = smpool.tile([P, 1], f32, name="mu2", tag="sm")
                nc.vector.tensor_mul(mu2[:], mu[:], mu[:])
                ve = smpool.tile([P, 1], f32, name="ve", tag="sm")
                nc.vector.scalar_tensor_tensor(
                    ve[:], mu2[:], -1.0, ex2[:],
                    mybir.AluOpType.mult, mybir.AluOpType.add,
                )
                nc.vector.tensor_scalar_add(ve[:], ve[:], EPS)
                sd = smpool.tile([P, 1], f32, name="sd", tag="sm")
                nc.scalar.sqrt(sd[:], ve[:])
                r = smpool.tile([P, 1], f32, name="r", tag="sm")
                nc.vector.reciprocal(r[:], sd[:])
                a = smpool.tile([P, 1], f32, name="a", tag="a")
                nc.vector.tensor_mul(a[:], r[:], prm[:, 9:10])
                s9 = smpool.tile([P, 1], f32, name="s9", tag="sm")
                nc.vector.tensor_reduce(
                    s9[:], prm[:, 0:9], mybir.AxisListType.X,
                    mybir.AluOpType.add,
                )
                am = smpool.tile([P, 1], f32, name="am", tag="sm")
                nc.vector.tensor_mul(am[:], a[:], mu[:])
                tb = smpool.tile([P, 1], f32, name="tb", tag="tb")
                nc.vector.scalar_tensor_tensor(
                    tb[:], am[:], -1.0, s9[:],
                    mybir.AluOpType.mult, mybir.AluOpType.mult,
                )
                return a, tb

            # ---- depthwise on PE via diagonal fp8 weights; 4 DoubleRow
            # tap-pairs + 1 single matmul per chunk.  Evictions run on the
            # vector engine as fused (psum*a + tb) tensor_scalar, one per
            # 4-chunk psum group.
            for j, t in enumerate(TILE_ORDER):
                x3 = x8t[t]
                scr = accpool.tile([P, HW], f16, name="acc", tag="acc")
                with tc.high_priority(offset=None if j < 2 else 0):
                    a, tb = stats(t, x3[:, 1:H + 1, 1:W + 1],
                                  scr.rearrange("p (h w) -> p h w", h=H))
                groups = [
                    psum.tile([P, 1024], f32, name="bank", tag="bank")
                    for _ in range(4)
                ]
                for bi, blk in enumerate(DW_BLOCKS):
                    first, last = bi == 0, bi == len(DW_BLOCKS) - 1
                    if len(blk) == 2:
                        (dyA, dxA), (dyB, dxB) = blk
                        lhsT = dg_sb[:, t * 1152 + bi * 256:
                                     t * 1152 + (bi + 1) * 256].rearrange(
                            "p (two m) -> p two m", two=2)
                        stride = (dyB - dyA) * IMGC + (dxB - dxA)
                        for ch in range(NCHUNK):
                            base = x3[:, ch * 8 + dyA:ch * 8 + dyA + 8,
                                      dxA:dxA + W]
                            rhs = AP(base.tensor, base.offset,
                                     [list(base.ap[0]), [stride, 2],
                                      [IMGC, 8], [1, W]])
                            nc.tensor.matmul(
                                groups[ch // 2][:, (ch % 2) * 512:
                                                (ch % 2) * 512 + 512],
                                lhsT, rhs, start=first, stop=last,
                                perf_mode=DR)
                    else:
                        (dy, dx), = blk
                        lhsT = dg_sb[:, t * 1152 + 1024:t * 1152 + 1152]
                        for ch in range(NCHUNK):
                            rhs = x3[:, ch * 8 + dy:ch * 8 + dy + 8, dx:dx + W]
                            nc.tensor.matmul(
                                groups[ch // 2][:, (ch % 2) * 512:
                                                (ch % 2) * 512 + 512],
                                lhsT, rhs, start=first, stop=last)
                for gi in range(4):
                    nc.vector.tensor_scalar(
                        v4[:, t, 1 + gi * 16:1 + gi * 16 + 16, 1:W + 1],
                        groups[gi].rearrange("p (h w) -> p h w", h=16),
                        a[:], tb[:],
                        mybir.AluOpType.mult, mybir.AluOpType.add,
                    )
                _reflect_borders(nc, v4[:, t])

            # bias constant, exact fp32 on the vector engine:
            # A[co] = sum_ci S[co,ci]*b[ci];  AB[:,co_t] = A + conv_b
            AB = prmpool.tile([P, CT], f32, name="AB", tag="ab")
            ascr = ypool.tile([P, 512], f32, name="ascr", tag="ascr", bufs=1)
            for co_t in range(CT):
                at = smpool.tile([P, 1], f32, name="at", tag="at")
                nc.vector.scalar_tensor_tensor(
                    ascr[:], s2_sb[:, co_t * 512:(co_t + 1) * 512], 1.0,
                    brow_sb[:],
                    mybir.AluOpType.mult, mybir.AluOpType.mult,
                    accum_out=at[:],
                )
                nc.vector.tensor_add(AB[:, co_t:co_t + 1], at[:],
                                     cb_sb[:, co_t:co_t + 1])

            # ---- dense 3x3: fp8 DoubleRow, pair dim = two ci tiles ------
            out_rr = (nc.sync, nc.scalar, nc.gpsimd)
            n_out = 0
            for co in range(CT):
                groups = [
                    psum.tile([P, 1024], f32, name="bank", tag="bank")
                    for _ in range(4)
                ]
                for ji, pi in enumerate(PAIR_ORDER):
                    for tap, dy, dx in _taps():
                        idx = (co * 2 + ji) * 9 + tap
                        w_view = wt_sb[:, idx * 256:(idx + 1) * 256].rearrange(
                            "p (two m) -> p two m", two=2)
                        for ch in range(NCHUNK):
                            rhs = v4[:, 2 * pi:2 * pi + 2,
                                     ch * 8 + dy:ch * 8 + dy + 8, dx:dx + W]
                            nc.tensor.matmul(
                                groups[ch // 2][:, (ch % 2) * 512:
                                                (ch % 2) * 512 + 512],
                                w_view, rhs,
                                start=(ji == 0 and tap == 0),
                                stop=(ji == 1 and tap == 8),
                                perf_mode=DR,
                            )
                for gi in range(4):
                    o = opool.tile([P, 1024], f32, name="o", tag="o")
                    nc.vector.tensor_scalar(
                        o[:], groups[gi][:], OUT_SCALE, AB[:, co:co + 1],
                        mybir.AluOpType.mult, mybir.AluOpType.add,
                    )
                    for hf in range(2):
                        out_rr[n_out % 3].dma_start(
                            out_ext[co * P:(co + 1) * P,
                                    gi * 1024 + hf * 512:
                                    gi * 1024 + (hf + 1) * 512],
                            o[:, hf * 512:(hf + 1) * 512],
                        )
                        n_out += 1

    nc.compile()
    _dedup_ldweights(nc)
    return nc


def kernel(x, w_spatial, w_pointwise, bias, conv_w, conv_b):
    global LAST_EXEC_NS
    if "nc" not in _CACHE:
        _CACHE["nc"] = _build()
    nc = _CACHE["nc"]

    xf = np.asarray(x, dtype=np.float32).astype(np.float16)
    xpad = np.pad(xf, ((0, 0), (0, 0), (1, 1), (1, 1)), mode="reflect")
    ws = np.asarray(w_spatial, dtype=np.float32).reshape(B, C, 9)
    wp = np.asarray(w_pointwise, dtype=np.float32).reshape(B, C)
    bi = np.asarray(bias, dtype=np.float32).reshape(B, C)
    cw = np.asarray(conv_w, dtype=np.float32)
    cb = np.asarray(conv_b, dtype=np.float32)

    # shared dense weights, fp8, emission-order blocks [p, ko, m]:
    # wt[p, ((co*2+j)*9+tap)*256 + ko*128 + m]
    #   = fp8(conv_w[co*128+m, ci*128+p, tap] * 2^M_SC), ci = CI_PAIRS[pi][ko]
    w8 = (cw.reshape(C, C, 9) * (2.0 ** M_SC)).astype(E4)
    wt = np.zeros((P, 72 * 256), dtype=E4)
    w8v = w8.view(np.uint8)
    wtv = wt.view(np.uint8)
    for co in range(CT):
        for ji, pi in enumerate(PAIR_ORDER):
            for tap in range(9):
                idx = (co * 2 + ji) * 9 + tap
                for ko in range(2):
                    ci_t = CI_PAIRS[pi][ko]
                    blk = w8v[co * P:(co + 1) * P,
                              ci_t * P:(ci_t + 1) * P, tap].T
                    wtv[:, idx * 256 + ko * P:idx * 256 + (ko + 1) * P] = blk

    # bias-constant operands: s2[p, co_t*512 + ci] = S[co_t*128+p, ci]
    S = cw.sum(axis=(2, 3))                      # [co, ci]
    s2 = np.zeros((P, 4 * 512), dtype=np.float32)
    for co_t in range(CT):
        s2[:, co_t * 512:(co_t + 1) * 512] = S[co_t * P:(co_t + 1) * P, :]
    cbl = np.ascontiguousarray(cb.reshape(CT, P).T)  # [p, CT]

    in_maps = []
    for b in range(B):
        # fp8 padded x, rows stored at stride 72
        x8 = np.zeros((C, PADH, IMGC), dtype=E4)
        x8[:, :, 0:PADW] = xpad[b].astype(E4)
        # per-channel params; the fp8-rounded taps (scaled 2^S_SC) go in so
        # the mu*s9 correction matches the diag weights exactly
        wsq8 = (ws[b] * (2.0 ** S_SC)).astype(E4)
        prm = np.zeros((CT, P, 11), dtype=np.float32)
        prm[:, :, 0:9] = wsq8.astype(np.float32).reshape(CT, P, 9)
        prm[:, :, 9] = wp[b].reshape(CT, P)
        prm = np.concatenate(
            [prm.transpose(1, 0, 2).reshape(P, CT * 11), cbl], axis=1)
        prm = np.ascontiguousarray(prm)

        # diagonal fp8 depthwise weights, one 1152-col block per tile
        dg = np.zeros((P, 4 * 1152), dtype=E4)
        idxp = np.arange(P)
        for t in range(CT):
            wsq = wsq8[t * P:(t + 1) * P]
            for bi2, blk in enumerate(DW_BLOCKS):
                if len(blk) == 2:
                    for ko, (dy, dx) in enumerate(blk):
                        dg[idxp, t * 1152 + bi2 * 256 + ko * P + idxp] = \
                            wsq[:, dy * 3 + dx]
                else:
                    (dy, dx), = blk
                    dg[idxp, t * 1152 + 1024 + idxp] = wsq[:, dy * 3 + dx]

        brow = np.ascontiguousarray(
            np.broadcast_to(bi[b][None, :], (P, C)).astype(np.float32))

        in_maps.append({
            "x8": x8.reshape(C, IMG),
            "wt": wt,
            "dg": dg,
            "s2": s2,
            "brow": brow,
            "prm": prm,
        })

    res = run_bass_kernel_spmd(
        nc, in_maps, list(range(N_CORES)), trace=_TRACE
    )
    LAST_EXEC_NS = res.exec_time_ns
    out = np.stack([res.results[b]["out"].reshape(C, H, W) for b in range(B)])
    return out


# revision 13
# speedup vs baseline: 3.6110x; 1.0095x over previous
"""Trainium2 Bass kernel for AdaConv2d (instance-norm + per-sample dynamic
depthwise 3x3 conv + per-channel scale/bias + shared dense 3x3 conv, reflect
padding everywhere).

Data-parallel over batch: 8 samples -> 8 NeuronCores, one sample per core.

Math (per sample, per channel c):
    xn   = (x - mu_c) * rsqrt(var_c + eps)
    mid  = wp_c * depthwise3x3(reflect_pad(xn); ws_c) + b_c
         = v_c + b_c    with v = a*dw(x) - a*mu*s9,  a = wp*rsqrt(var+eps)
    out  = dense3x3(reflect_pad(mid); conv_w) + conv_b
         = dense3x3(reflect_pad(v); conv_w) + S@b + conv_b
      where S[co,ci] = sum_taps conv_w  (reflect padding makes the per-channel
      constant b contribute exactly S@b at every output pixel).

The v split is what makes fp8 viable: v has std ~0.0075 (vs mid's ~0.05
dominated by the per-channel constant b), so quantizing v*2^9 to fp8e4m3
keeps the dense-conv error ~1e-2 relative (gate is 2e-2).  The dense conv
runs as fp8 DoubleRow matmuls: each matmul contracts 256 = 2 channel tiles
(the pair dim of the moving AP strides between two v images that live in
one SBUF tile), at ~1.7x the bf16 rate.  The constant S@b + conv_b is
computed exactly in fp32 on the vector engine (elementwise mult with a
host-broadcast b row + row-reduce) and added as the eviction bias.

All four depthwise tiles run on the tensor engine as diagonal-fp8-weight
matmuls, DoubleRow-pairing two taps per matmul (pair dim = the 2-byte or
2-row shift between the taps' windows).  The vector engine only does the
psum evictions (fused scale+bias tensor_scalar), stats small-ops, reflect
borders and the bias constant; the scalar engine runs the four stats
passes.  PSUM cycles as 4-bank [128,2048] group tiles so the scheduler
keeps each weight block's 8 chunk matmuls contiguous (one LDWEIGHTS per
block instead of per matmul).
"""

import os
import sys
import types

import numpy as np
import ml_dtypes

B, C, H, W = 8, 512, 64, 64
KS = 3
EPS = 1e-5
N_CORES = 8
P = 128
CT = C // P            # 4 channel tiles
PADH = H + 2           # 66
PADW = W + 2           # 66
IMGC = 72              # stored row stride (16B-aligned images: 66*72 = 4752)
IMG = PADH * IMGC      # 4752
HW = H * W             # 4096
NCHUNK = 8             # 8-row output chunks -> one psum bank each
TILE_ORDER = (2, 3, 0, 1)  # depthwise production order (dense eats 2,3 first)
CI_PAIRS = ((0, 1), (2, 3))
PAIR_ORDER = (1, 0)    # dense contraction: pair (2,3) first (ready early)
K_SC = 9               # v scale 2^9
M_SC = 9               # dense weight scale 2^9
S_SC = 9               # depthwise diag weight scale 2^9
OUT_SCALE = float(2.0 ** (-(K_SC + M_SC)))
# depthwise tap blocks: 4 DoubleRow pairs + 1 single
DW_BLOCKS = (((0, 0), (0, 2)), ((1, 0), (1, 2)), ((2, 0), (2, 2)),
             ((0, 1), (2, 1)), ((1, 1),))

E4 = ml_dtypes.float8_e4m3


def _install_ntff_hook():
    """Register the NTFF profiling hook that concourse expects under axon
    (missing antenv.axon_hooks module in this image)."""
    if "antenv.axon_hooks" in sys.modules:
        return
    try:
        mod = types.ModuleType("antenv.axon_hooks")
        holder = [None]
        mod.set_axon_ntff_profile_hook = lambda h: holder.__setitem__(0, h)
        mod.get_axon_ntff_profile_hook = lambda: holder[0]
        sys.modules["antenv.axon_hooks"] = mod
        from trn_agent_boot.trn_boot import _ntff_profile_via_ctypes

        hook = _ntff_profile_via_ctypes("/opt/axon/libaxon_pjrt.so")
        mod.set_axon_ntff_profile_hook(hook)
    except Exception:
        sys.modules.pop("antenv.axon_hooks", None)


_TRACE = os.environ.get("BASS_KERNEL_TRACE") == "1"
if _TRACE:
    _install_ntff_hook()

import concourse.tile as tile
from concourse import bacc, mybir
import concourse.bass_utils as bass_utils
from concourse.bass_utils import run_bass_kernel_spmd
from concourse.ap import AP

if _TRACE:
    bass_utils.upload_artifacts = lambda d: d

LAST_EXEC_NS = None
_CACHE = {}


def _taps():
    for tap in range(KS * KS):
        yield tap, tap // KS, tap % KS


def _reflect_borders(nc, img3):
    """Fill the 1-wide reflect border of a [128, 66, 72] image whose
    interior [1:65, 1:65] is already populated (cols first, then full rows
    so the corners come out as reflect-of-reflect, matching np.pad)."""
    nc.vector.tensor_copy(img3[:, 1:H + 1, 0:1], img3[:, 1:H + 1, 2:3])
    nc.vector.tensor_copy(img3[:, 1:H + 1, PADW - 1:PADW],
                          img3[:, 1:H + 1, PADW - 3:PADW - 2])
    nc.vector.tensor_copy(img3[:, 0:1, 0:PADW], img3[:, 2:3, 0:PADW])
    nc.vector.tensor_copy(img3[:, PADH - 1:PADH, 0:PADW],
                          img3[:, PADH - 3:PADH - 2, 0:PADW])


def _dedup_ldweights(nc):
    """Drop InstLdweights whose weights AP is identical to the previous
    weight load on the PE stream (bacc splits every matmul into LDW+MM;
    with one weight block reused across 8 chunk matmuls, 7 of 8 loads are
    redundant and serialize with the matmuls).  LDWs carrying semaphore
    waits/updates are kept."""
    n_removed = 0
    for f in nc.m.functions:
        for bb in f.blocks:
            insts = bb.instructions
            keep = []
            last_key = None
            for inst in insts:
                tn = type(inst).__name__
                if tn == "InstLdweights":
                    si = inst.sync_info
                    has_sync = si is not None and (
                        len(si.on_wait) > 0 or len(si.on_update) > 0
                    )
                    key = repr(inst.ins[0])
                    if key == last_key and not has_sync:
                        n_removed += 1
                        continue
                    last_key = key
                elif tn == "InstMatmult":
                    if getattr(inst, "is_transpose", False):
                        last_key = None
                keep.append(inst)
            if len(keep) != len(insts):
                bb.instructions = keep
    return n_removed


def _build():
    nc = bacc.Bacc("TRN2", target_bir_lowering=False, debug=False,
                   num_devices=N_CORES)
    f32 = mybir.dt.float32
    f16 = mybir.dt.float16
    f8 = mybir.dt.float8e4
    DR = mybir.MatmulPerfMode.DoubleRow

    x8_in = nc.dram_tensor("x8", [C, IMG], f8, kind="ExternalInput").ap()
    wt_in = nc.dram_tensor("wt", [P, 72 * 256], f8, kind="ExternalInput").ap()
    dg_in = nc.dram_tensor("dg", [P, 4 * 1152], f8, kind="ExternalInput").ap()
    s2_in = nc.dram_tensor("s2", [P, 4 * 512], f32, kind="ExternalInput").ap()
    brow_in = nc.dram_tensor("brow", [P, 512], f32, kind="ExternalInput").ap()
    prm_in = nc.dram_tensor("prm", [P, CT * 11 + CT], f32,
                            kind="ExternalInput").ap()
    out_ext = nc.dram_tensor("out", [C, HW], f32, kind="ExternalOutput").ap()

    with tile.TileContext(nc) as tc:
        with (
            tc.tile_pool(name="wpool", bufs=1) as wpool,
            tc.tile_pool(name="xpool", bufs=2) as xpool,
            tc.tile_pool(name="vpool", bufs=1) as vpool,
            tc.tile_pool(name="accpool", bufs=2) as accpool,
            tc.tile_pool(name="ypool", bufs=4) as ypool,
            tc.tile_pool(name="smpool", bufs=8) as smpool,
            tc.tile_pool(name="prmpool", bufs=4) as prmpool,
            tc.tile_pool(name="opool", bufs=4) as opool,
            tc.tile_pool(name="psum", bufs=4, space="PSUM") as psum,
        ):
            # ---- input DMAs over the three rings (SP, ACT, POOL) --------
            # scalar ring: dw diag weights + params + first half of the
            # dense weights (co-major blocks: co0/co1 land first).
            dg_sb = wpool.tile([P, 4 * 1152], f8, name="dg_sb", tag="dg")
            for t in TILE_ORDER[:2]:
                nc.scalar.dma_start(dg_sb[:, t * 1152:(t + 1) * 1152],
                                    dg_in[:, t * 1152:(t + 1) * 1152])
            prm_all = prmpool.tile([P, CT * 11 + CT], f32, name="prm_all",
                                   tag="prm")
            nc.scalar.dma_start(prm_all[:], prm_in[:])

            # fp8 x in production order: t2 (quartered, earliest matmuls),
            # t3 + t1 on the sync ring; t0 on the scalar ring in parallel.
            x8t = {}
            xtiles = {}
            for t in TILE_ORDER:
                xp = xpool.tile([P, IMG], f8, name=f"x8_{t}", tag=f"x8{t}",
                                bufs=1)
                xtiles[t] = xp
                x8t[t] = xp.rearrange("p (h w) -> p h w", h=PADH)
            qh = 18 * IMGC
            for qi in range(4):
                t = TILE_ORDER[0]
                lo, hi = qi * qh, min((qi + 1) * qh, IMG)
                nc.sync.dma_start(xtiles[t][:, lo:hi],
                                  x8_in[t * P:t * P + P, lo:hi])
            hh8 = (PADH // 2) * IMGC
            for t in (TILE_ORDER[1], TILE_ORDER[3]):
                nc.sync.dma_start(xtiles[t][:, 0:hh8],
                                  x8_in[t * P:t * P + P, 0:hh8])
                nc.sync.dma_start(xtiles[t][:, hh8:IMG],
                                  x8_in[t * P:t * P + P, hh8:IMG])
            t = TILE_ORDER[2]
            nc.scalar.dma_start(xtiles[t][:, 0:hh8],
                                x8_in[t * P:t * P + P, 0:hh8])
            nc.scalar.dma_start(xtiles[t][:, hh8:IMG],
                                x8_in[t * P:t * P + P, hh8:IMG])
            for t in TILE_ORDER[2:]:
                nc.scalar.dma_start(dg_sb[:, t * 1152:(t + 1) * 1152],
                                    dg_in[:, t * 1152:(t + 1) * 1152])
            wt_sb = wpool.tile([P, 72 * 256], f8, name="wt_sb", tag="wt")
            nc.scalar.dma_start(wt_sb[:, 0:36 * 256], wt_in[:, 0:36 * 256])
            nc.sync.dma_start(wt_sb[:, 36 * 256:72 * 256],
                              wt_in[:, 36 * 256:72 * 256])

            # gpsimd ring: bias-constant operands
            s2_sb = wpool.tile([P, 4 * 512], f32, name="s2_sb", tag="s2")
            nc.gpsimd.dma_start(s2_sb[:], s2_in[:])
            brow_sb = wpool.tile([P, 512], f32, name="brow_sb", tag="brow")
            nc.gpsimd.dma_start(brow_sb[:], brow_in[:])

            prms = [prm_all[:, t * 11:(t + 1) * 11] for t in range(CT)]
            cb_sb = prm_all[:, CT * 11:CT * 11 + CT]

            # the four v images (fp8, 2^9-scaled varying part of mid) in one
            # tile so the dense DoubleRow pair dim can stride between them.
            v4 = vpool.tile([P, CT, PADH, IMGC], f8, name="v4", tag="v4")

            def stats(t, xin, scratch3):
                """mean/var of the tile -> per-channel affine (a, tb2) with
                v = a*acc + tb2 (the 2^k scale is folded into prm's wp).
                scratch3 receives the squares (overwritten later)."""
                prm = prms[t]
                sqs = smpool.tile([P, 1], f32, name="sqs", tag="sm")
                ms = smpool.tile([P, 1], f32, name="ms", tag="sm")
                sscr = ypool.tile([P, HW], f16, name="y", tag="y")
                sscr3 = sscr.rearrange("p (h w) -> p h w", h=H)
                nc.scalar.activation(
                    scratch3, xin,
                    mybir.ActivationFunctionType.Square, accum_out=sqs[:],
                )
                nc.scalar.activation(
                    sscr3, xin,
                    mybir.ActivationFunctionType.Identity, accum_out=ms[:],
                )
                mu = smpool.tile([P, 1], f32, name="mu", tag="sm")
                nc.vector.tensor_scalar_mul(mu[:], ms[:], 1.0 / HW)
                ex2 = smpool.tile([P, 1], f32, name="ex2", tag="sm")
                nc.vector.tensor_scalar_mul(ex2[:], sqs[:], 1.0 / HW)
                mu2 = smpool.tile([P, 1], f32, name="mu2", tag="sm")
                nc.vector.tensor_mul(mu2[:], mu[:], mu[:])
                ve = smpool.tile([P, 1], f32, name="ve", tag="sm")
                nc.vector.scalar_tensor_tensor(
                    ve[:], mu2[:], -1.0, ex2[:],
                    mybir.AluOpType.mult, mybir.AluOpType.add,
                )
                nc.vector.tensor_scalar_add(ve[:], ve[:], EPS)
                sd = smpool.tile([P, 1], f32, name="sd", tag="sm")
                nc.scalar.sqrt(sd[:], ve[:])
                r = smpool.tile([P, 1], f32, name="r", tag="sm")
                nc.vector.reciprocal(r[:], sd[:])
                a = smpool.tile([P, 1], f32, name="a", tag="a")
                nc.vector.tensor_mul(a[:], r[:], prm[:, 9:10])
                s9 = smpool.tile([P, 1], f32, name="s9", tag="sm")
                nc.vector.tensor_reduce(
                    s9[:], prm[:, 0:9], mybir.AxisListType.X,
                    mybir.AluOpType.add,
                )
                am = smpool.tile([P, 1], f32, name="am", tag="sm")
                nc.vector.tensor_mul(am[:], a[:], mu[:])
                tb = smpool.tile([P, 1], f32, name="tb", tag="tb")
                nc.vector.scalar_tensor_tensor(
                    tb[:], am[:], -1.0, s9[:],
                    mybir.AluOpType.mult, mybir.AluOpType.mult,
                )
                return a, tb

            # ---- depthwise on PE via diagonal fp8 weights; 4 DoubleRow
            # tap-pairs + 1 single matmul per chunk.  Evictions run on the
            # vector engine as fused (psum*a + tb) tensor_scalar, one per
            # 4-chunk psum group.
            for j, t in enumerate(TILE_ORDER):
                x3 = x8t[t]
                scr = accpool.tile([P, HW], f16, name="acc", tag="acc")
                with tc.high_priority(offset=None if j < 2 else 0):
                    a, tb = stats(t, x3[:, 1:H + 1, 1:W + 1],
                                  scr.rearrange("p (h w) -> p h w", h=H))
                groups = [
                    psum.tile([P, 1024], f32, name="bank", tag="bank")
                    for _ in range(4)
                ]
                for bi, blk in enumerate(DW_BLOCKS):
                    first, last = bi == 0, bi == len(DW_BLOCKS) - 1
                    if len(blk) == 2:
                        (dyA, dxA), (dyB, dxB) = blk
                        lhsT = dg_sb[:, t * 1152 + bi * 256:
                                     t * 1152 + (bi + 1) * 256].rearrange(
                            "p (two m) -> p two m", two=2)
                        stride = (dyB - dyA) * IMGC + (dxB - dxA)
                        for ch in range(NCHUNK):
                            base = x3[:, ch * 8 + dyA:ch * 8 + dyA + 8,
                                      dxA:dxA + W]
                            rhs = AP(base.tensor, base.offset,
                                     [list(base.ap[0]), [stride, 2],
                                      [IMGC, 8], [1, W]])
                            nc.tensor.matmul(
                                groups[ch // 2][:, (ch % 2) * 512:
                                                (ch % 2) * 512 + 512],
                                lhsT, rhs, start=first, stop=last,
                                perf_mode=DR)
                    else:
                        (dy, dx), = blk
                        lhsT = dg_sb[:, t * 1152 + 1024:t * 1152 + 1152]
                        for ch in range(NCHUNK):
                            rhs = x3[:, ch * 8 + dy:ch * 8 + dy + 8, dx:dx + W]
                            nc.tensor.matmul(
                                groups[ch // 2][:, (ch % 2) * 512:
                                                (ch % 2) * 512 + 512],
                                lhsT, rhs, start=first, stop=last)
                for gi in range(4):
                    nc.vector.tensor_scalar(
                        v4[:, t, 1 + gi * 16:1 + gi * 16 + 16, 1:W + 1],
                        groups[gi].rearrange("p (h w) -> p h w", h=16),
                        a[:], tb[:],
                        mybir.AluOpType.mult, mybir.AluOpType.add,
                    )
                _reflect_borders(nc, v4[:, t])

            # bias constant, exact fp32 on the vector engine:
            # A[co] = sum_ci S[co,ci]*b[ci];  AB[:,co_t] = A + conv_b
            AB = prmpool.tile([P, CT], f32, name="AB", tag="ab")
            ascr = ypool.tile([P, 512], f32, name="ascr", tag="ascr", bufs=1)
            for co_t in range(CT):
                at = smpool.tile([P, 1], f32, name="at", tag="at")
                nc.vector.scalar_tensor_tensor(
                    ascr[:], s2_sb[:, co_t * 512:(co_t + 1) * 512], 1.0,
                    brow_sb[:],
                    mybir.AluOpType.mult, mybir.AluOpType.mult,
                    accum_out=at[:],
                )
                nc.vector.tensor_add(AB[:, co_t:co_t + 1], at[:],
                                     cb_sb[:, co_t:co_t + 1])

            # ---- dense 3x3: fp8 DoubleRow, pair dim = two ci tiles ------
            out_rr = (nc.sync, nc.scalar, nc.gpsimd)
            n_out = 0
            for co in range(CT):
                groups = [
                    psum.tile([P, 1024], f32, name="bank", tag="bank")
                    for _ in range(4)
                ]
                for ji, pi in enumerate(PAIR_ORDER):
                    for tap, dy, dx in _taps():
                        idx = (co * 2 + ji) * 9 + tap
                        w_view = wt_sb[:, idx * 256:(idx + 1) * 256].rearrange(
                            "p (two m) -> p two m", two=2)
                        for ch in range(NCHUNK):
                            rhs = v4[:, 2 * pi:2 * pi + 2,
                                     ch * 8 + dy:ch * 8 + dy + 8, dx:dx + W]
                            nc.tensor.matmul(
                                groups[ch // 2][:, (ch % 2) * 512:
                                                (ch % 2) * 512 + 512],
                                w_view, rhs,
                                start=(ji == 0 and tap == 0),
                                stop=(ji == 1 and tap == 8),
                                perf_mode=DR,
                            )
                for gi in range(4):
                    o = opool.tile([P, 1024], f32, name="o", tag="o")
                    nc.vector.tensor_scalar(
                        o[:], groups[gi][:], OUT_SCALE, AB[:, co:co + 1],
                        mybir.AluOpType.mult, mybir.AluOpType.add,
                    )
                    for hf in range(2):
                        out_rr[n_out % 3].dma_start(
                            out_ext[co * P:(co + 1) * P,
                                    gi * 1024 + hf * 512:
                                    gi * 1024 + (hf + 1) * 512],
                            o[:, hf * 512:(hf + 1) * 512],
                        )
                        n_out += 1

    nc.compile()
    _dedup_ldweights(nc)
    return nc


def kernel(x, w_spatial, w_pointwise, bias, conv_w, conv_b):
    global LAST_EXEC_NS
    if "nc" not in _CACHE:
        _CACHE["nc"] = _build()
    nc = _CACHE["nc"]

    xf = np.asarray(x, dtype=np.float32).astype(np.float16)
    xpad = np.pad(xf, ((0, 0), (0, 0), (1, 1), (1, 1)), mode="reflect")
    ws = np.asarray(w_spatial, dtype=np.float32).reshape(B, C, 9)
    wp = np.asarray(w_pointwise, dtype=np.float32).reshape(B, C)
    bi = np.asarray(bias, dtype=np.float32).reshape(B, C)
    cw = np.asarray(conv_w, dtype=np.float32)
    cb = np.asarray(conv_b, dtype=np.float32)

    # shared dense weights, fp8, emission-order blocks [p, ko, m]:
    # wt[p, ((co*2+j)*9+tap)*256 + ko*128 + m]
    #   = fp8(conv_w[co*128+m, ci*128+p, tap] * 2^M_SC), ci = CI_PAIRS[pi][ko]
    w8 = (cw.reshape(C, C, 9) * (2.0 ** M_SC)).astype(E4)
    wt = np.zeros((P, 72 * 256), dtype=E4)
    w8v = w8.view(np.uint8)
    wtv = wt.view(np.uint8)
    for co in range(CT):
        for ji, pi in enumerate(PAIR_ORDER):
            for tap in range(9):
                idx = (co * 2 + ji) * 9 + tap
                for ko in range(2):
                    ci_t = CI_PAIRS[pi][ko]
                    blk = w8v[co * P:(co + 1) * P,
                              ci_t * P:(ci_t + 1) * P, tap].T
                    wtv[:, idx * 256 + ko * P:idx * 256 + (ko + 1) * P] = blk

    # bias-constant operands: s2[p, co_t*512 + ci] = S[co_t*128+p, ci]
    S = cw.sum(axis=(2, 3))                      # [co, ci]
    s2 = np.zeros((P, 4 * 512), dtype=np.float32)
    for co_t in range(CT):
        s2[:, co_t * 512:(co_t + 1) * 512] = S[co_t * P:(co_t + 1) * P, :]
    cbl = np.ascontiguousarray(cb.reshape(CT, P).T)  # [p, CT]

    in_maps = []
    for b in range(B):
        # fp8 padded x, rows stored at stride 72
        x8 = np.zeros((C, PADH, IMGC), dtype=E4)
        x8[:, :, 0:PADW] = xpad[b].astype(E4)
        # per-channel params; the fp8-rounded taps (scaled 2^S_SC) go in so
        # the mu*s9 correction matches the diag weights exactly
        wsq8 = (ws[b] * (2.0 ** S_SC)).astype(E4)
        prm = np.zeros((CT, P, 11), dtype=np.float32)
        prm[:, :, 0:9] = wsq8.astype(np.float32).reshape(CT, P, 9)
        prm[:, :, 9] = wp[b].reshape(CT, P)
        prm = np.concatenate(
            [prm.transpose(1, 0, 2).reshape(P, CT * 11), cbl], axis=1)
        prm = np.ascontiguousarray(prm)

        # diagonal fp8 depthwise weights, one 1152-col block per tile
        dg = np.zeros((P, 4 * 1152), dtype=E4)
        idxp = np.arange(P)
        for t in range(CT):
            wsq = wsq8[t * P:(t + 1) * P]
            for bi2, blk in enumerate(DW_BLOCKS):
                if len(blk) == 2:
                    for ko, (dy, dx) in enumerate(blk):
                        dg[idxp, t * 1152 + bi2 * 256 + ko * P + idxp] = \
                            wsq[:, dy * 3 + dx]
                else:
                    (dy, dx), = blk
                    dg[idxp, t * 1152 + 1024 + idxp] = wsq[:, dy * 3 + dx]

        brow = np.ascontiguousarray(
            np.broadcast_to(bi[b][None, :], (P, C)).astype(np.float32))

        in_maps.append({
            "x8": x8.reshape(C, IMG),
            "wt": wt,
            "dg": dg,
            "s2": s2,
            "brow": brow,
            "prm": prm,
        })

    res = run_bass_kernel_spmd(
        nc, in_maps, list(range(N_CORES)), trace=_TRACE
    )
    LAST_EXEC_NS = res.exec_time_ns
    out = np.stack([res.results[b]["out"].reshape(C, H, W) for b in range(B)])
    return out
